# revision 50
# baseline (speedup 1.0000x reference)
"""Trainium2 Bass kernel for one transformer block (B=2, T=2048, C=768, H=12,
inner=3072, fp32 io, causal, post-norm residual).

Sharding: 8 cores, token-interleaved. Core c handles batch c//4, tokens
p::4 (p = c%4) of that batch - every core runs the IDENTICAL program
(SPMD), causality is data-driven via a per-core diagonal-sliver mask.

Design (341us f32r baseline -> 296us):
 - all matmul operands bf16 (weights, x, q/k/v, ctx, h, gelu out): halves
   HBM traffic, enables FWL fast weight load, no f32r 256-col floor.
   Residual stream (r1/r2) and LN math stay f32/f32r (rel err ~3e-3).
 - causality: score matmul for key chunk kc covers only queries >= 32*kc;
   only the 32-query diagonal sliver needs masking (mask is chunk-
   independent: key k_local visible iff k_local <= 4*q_local + p), so the
   DVE mask work is ~10x smaller than masking every score.
 - attention per head pair (pair hp shares kT chunk cc=hp): kc<8 scores
   use 64-row PE tiling (per-head array halves -> different PSUM banks);
   kc>=8 pack both heads' scores into one 512-f32 bank via the
   zero-padded-q trick. ctx chunks interleave with the kc>=8 scores so
   the PE rides the exp (ACT) stream without stalling; softmax
   denominators ride as a ones-column on V, reciprocals via the fast
   approx DVE op, partition-broadcast on the (pre-warmed) gpsimd engine.
 - Wo/W1/W2 stream from HBM during attention (SBUF freed by bf16);
   LN stats accumulate on the PE via ones-matmuls, batched to limit
   PE array-tiling mode switches.

Known pitfalls encoded here: one matmul output may not span PSUM banks;
two row-tiles must never write the same bank; custom DVE/gpsimd ops
require partition-0 operands; partition-shifted plain DVE ops need
32-aligned bases; gpsimd's first custom-op dispatch costs ~10us.
"""

import sys

if "/opt/trn_rl_repo" not in sys.path:
    sys.path.insert(0, "/opt/trn_rl_repo")

import numpy as np
import ml_dtypes

import concourse.bacc as bacc
import concourse.mybir as mybir
import concourse.tile as tile
from concourse.bass_utils import run_bass_kernel_spmd

F32 = mybir.dt.float32
F32R = mybir.dt.float32r
BF16 = mybir.dt.bfloat16
ACTF = mybir.ActivationFunctionType

B, T, C = 2, 2048, 768
H, DH = 12, 64
IN = 3072
CC = C // 128           # 6 channel chunks
TQ = 512                # tokens per core
KCN = T // 128          # 16 key chunks
ICN = IN // 128         # 24 inner chunks
HP = H // 2             # 6 head pairs
EPS = 1e-4
SCALE = 1.0 / np.sqrt(DH)

# param pack order in "prk" [128, CC, 8]
P_BQ, P_BK, P_BO, P_B2, P_L1S, P_L1B, P_L2S, P_L2B = range(8)


def _build_nc():
    nc = bacc.Bacc("TRN2", target_bir_lowering=False, debug=False,
                   enable_asserts=False, num_devices=8)
    d = {}
    d["xtq"] = nc.dram_tensor("xtq", [128, CC, TQ], F32, kind="ExternalInput").ap()
    d["xtqb"] = nc.dram_tensor("xtqb", [128, CC, TQ], BF16, kind="ExternalInput").ap()
    d["xtb"] = nc.dram_tensor("xtb", [128, CC, T], BF16, kind="ExternalInput").ap()
    for w, sh in (("wq", [128, CC, C]), ("wk", [128, CC, C]),
                  ("wv", [128, CC, C]), ("wo", [128, CC, C]),
                  ("w1", [128, CC, IN]), ("w2", [128, ICN, C])):
        d[w] = nc.dram_tensor(w, sh, BF16, kind="ExternalInput").ap()
    d["prk"] = nc.dram_tensor("prk", [128, CC, 8], F32, kind="ExternalInput").ap()
    d["b1p"] = nc.dram_tensor("b1p", [128, ICN], F32, kind="ExternalInput").ap()
    d["bvb"] = nc.dram_tensor("bvb", [128, C], F32, kind="ExternalInput").ap()
    d["msk"] = nc.dram_tensor("msk", [128, 2, 32], BF16, kind="ExternalInput").ap()
    d["ones"] = nc.dram_tensor("ones", [128, 128], F32R, kind="ExternalInput").ap()
    d["outT"] = nc.dram_tensor("outT", [128, CC, TQ], F32, kind="ExternalOutput").ap()

    with tile.TileContext(nc) as tc:
        _emit(nc, tc, d)
    nc.finalize()
    return nc


def _emit(nc, tc, d):
    # ---- persistent constants ------------------------------------------
    const = tc.alloc_tile_pool(name="const", bufs=1, side="left")
    ones_sb = const.tile([128, 128], F32R, name="ones_sb")
    prk_sb = const.tile([128, CC, 8], F32, name="prk_sb")
    b1p_sb = const.tile([128, ICN], F32, name="b1p_sb")
    bvb_sb = const.tile([128, C], F32, name="bvb_sb")
    msk_sb = const.tile([128, 2, 32], BF16, name="msk_sb")

    def prm(cc, pi):
        return prk_sb[:, cc, pi].unsqueeze(-1)  # [128,1]

    nc.sync.dma_start(out=prk_sb[:], in_=d["prk"][:])
    nc.sync.dma_start(out=msk_sb[:], in_=d["msk"][:])

    wo_pool = tc.alloc_tile_pool(name="wop", bufs=1, side="left")
    wo_t = wo_pool.tile([128, CC, C], BF16, name="wo_t")

    # right-side stack (LIFO): ctxT/xtq (A-C), qp/kv (A-B), xtqb/xtb (A)
    ctxT_pool = tc.alloc_tile_pool(name="ctxTp", bufs=1, side="right")
    ctxT_t = ctxT_pool.tile([128, CC, TQ], BF16, name="ctxT_t")
    xtq_pool = tc.alloc_tile_pool(name="xtq", bufs=1, side="right")
    xtq_t = xtq_pool.tile([128, CC, TQ], F32, name="xtq_t")
    qp_pool = tc.alloc_tile_pool(name="qp", bufs=1, side="right")
    qp_t = qp_pool.tile([128, CC, 2, TQ], BF16, name="qp_t")
    kv_pool = tc.alloc_tile_pool(name="kvp", bufs=1, side="right")
    kT_t = kv_pool.tile([128, CC, T], BF16, name="kT_t")
    v_t = kv_pool.tile([128, KCN, H, DH + 1], BF16, name="v_t")
    xtqb_pool = tc.alloc_tile_pool(name="xtqb", bufs=1, side="right")
    xtqb_t = xtqb_pool.tile([128, CC, TQ], BF16, name="xtqb_t")
    xtb_pool = tc.alloc_tile_pool(name="xtb", bufs=1, side="right")
    xtb_t = xtb_pool.tile([128, CC, T], BF16, name="xtb_t")

    # ==================== phase A: QKV ==================================
    with tc.tile_pool(name="wqkv", bufs=1, side="right") as wqkv, \
         tc.tile_pool(name="pqkv", bufs=2, space="PSUM") as pqkv:
        wq_t = wqkv.tile([128, CC, C], BF16, name="wq_t", tag="wq")
        wk_t = wqkv.tile([128, CC, C], BF16, name="wk_t", tag="wk")
        wv_t = wqkv.tile([128, CC, C], BF16, name="wv_t", tag="wv")
        nc.sync.dma_start(out=xtqb_t[:], in_=d["xtqb"][:])
        for i in range(3):
            nc.sync.dma_start(out=wq_t[:, 2 * i:2 * i + 2, :],
                              in_=d["wq"][:, 2 * i:2 * i + 2, :])
        nc.sync.dma_start(out=bvb_sb[:], in_=d["bvb"][:])
        nc.sync.dma_start(out=wv_t[:], in_=d["wv"][:])
        for i in range(3):
            nc.sync.dma_start(out=xtb_t[:, 2 * i:2 * i + 2, :],
                              in_=d["xtb"][:, 2 * i:2 * i + 2, :])
        nc.sync.dma_start(out=wk_t[:], in_=d["wk"][:])
        nc.sync.dma_start(out=ones_sb[:], in_=d["ones"][:])
        nc.sync.dma_start(out=b1p_sb[:], in_=d["b1p"][:])
        nc.sync.dma_start(out=xtq_t[:], in_=d["xtq"][:])

        # zero q staging (evicts below overwrite the live rows)
        nc.vector.memset(qp_t[:], 0.0)

        # warm up the gpsimd custom-instruction library off the critical
        # path (first dispatch pays a ~10us library-load/warmup penalty)
        gwarm = wqkv.tile([128, 16], F32R, name="gwarm", tag="gw")
        nc.gpsimd.partition_broadcast(gwarm[:], ones_sb[0:1, 0:16],
                                      channels=128)

        # q^T: per output chunk mc, heads 2mc (rows 0:64) / 2mc+1 (64:128)
        for mc in range(CC):
            ps = pqkv.tile([128, TQ], F32, name="ps_q", tag="pk", bufs=3)
            for kc in range(CC):
                nc.tensor.matmul(ps[:], wq_t[:, kc, mc * 128:(mc + 1) * 128],
                                 xtqb_t[:, kc, :],
                                 start=(kc == 0), stop=(kc == CC - 1))
            nc.vector.tensor_scalar_add(
                qp_t[0:64, mc, 0, :], ps[0:64, :], prm(mc, P_BQ)[0:64, :])
            nc.vector.tensor_scalar_add(
                qp_t[64:128, mc, 1, :], ps[64:128, :], prm(mc, P_BQ)[64:128, :])

        # v rows (token-major) + ones column for softmax denominators;
        # before k so the heavy v evicts drain while k matmuls run
        nc.vector.memset(v_t[:, :, :, DH], 1.0)
        for tch in range(KCN):
            ps1 = pqkv.tile([128, 512], F32, name="ps_v1", tag="pv1")
            ps2 = pqkv.tile([128, 256], F32, name="ps_v2", tag="pv2")
            for kc in range(CC):
                xsl = xtb_t[:, kc, tch * 128:(tch + 1) * 128]
                nc.tensor.matmul(ps1[:], xsl, wv_t[:, kc, 0:512],
                                 start=(kc == 0), stop=(kc == CC - 1))
                nc.tensor.matmul(ps2[:], xsl, wv_t[:, kc, 512:C],
                                 start=(kc == 0), stop=(kc == CC - 1))
            nc.vector.tensor_add(
                v_t[:, tch, 0:8, 0:DH],
                ps1[:].rearrange("p (h d) -> p h d", d=DH),
                bvb_sb[:, 0:512].rearrange("p (h d) -> p h d", d=DH))
            nc.vector.tensor_add(
                v_t[:, tch, 8:H, 0:DH],
                ps2[:].rearrange("p (h d) -> p h d", d=DH),
                bvb_sb[:, 512:C].rearrange("p (h d) -> p h d", d=DH))

        # k^T for the full 2048 keys, chunk mc ascending (attention pair 0
        # needs chunk 0 first)
        for mc in range(CC):
            for tb in range(4):
                ps = pqkv.tile([128, 512], F32, name="ps_k", tag="pk", bufs=3)
                for kc in range(CC):
                    nc.tensor.matmul(
                        ps[:], wk_t[:, kc, mc * 128:(mc + 1) * 128],
                        xtb_t[:, kc, tb * 512:(tb + 1) * 512],
                        start=(kc == 0), stop=(kc == CC - 1))
                nc.vector.tensor_scalar_add(
                    kT_t[:, mc, tb * 512:(tb + 1) * 512], ps[:],
                    prm(mc, P_BK))

    xtb_pool.release()
    xtqb_pool.release()

    # prefetch weights for later phases while attention runs (the pool
    # reuses the SBUF space xtb/xtqb just released)
    nc.sync.dma_start(out=wo_t[:], in_=d["wo"][:])
    wmlp_pool = tc.alloc_tile_pool(name="wmlp", bufs=1, side="left")
    w1_t = wmlp_pool.tile([128, CC, IN], BF16, name="w1_t")
    w2_t = wmlp_pool.tile([128, ICN, C], BF16, name="w2_t")

    # ==================== phase B: attention ============================
    with tc.tile_pool(name="attnp", bufs=4, side="right") as apool, \
         tc.tile_pool(name="psc", bufs=3, space="PSUM") as psc, \
         tc.tile_pool(name="pctx", bufs=1, space="PSUM") as pctx:

        def _score_chunk(hp, kc):
            """Scores + exp + mask for one key chunk. kc<8 uses 64-row PE
            tiling (both heads on independent array halves, different PSUM
            banks); kc>=8 one packed full-array matmul."""
            cc = hp
            q0 = 32 * kc
            w = TQ - q0
            ps = psc.tile([128, 2 * TQ], F32, name="ps_s", tag="s")
            et = apool.tile([128, 2 * TQ], BF16, name="et", tag="e",
                            bufs=KCN)
            if kc < 8:
                for j in range(2):
                    r0, off = j * DH, q0 if j == 0 else TQ
                    nc.tensor.matmul(
                        ps[:, off:off + w],
                        kT_t[r0:r0 + DH, cc, kc * 128:(kc + 1) * 128],
                        qp_t[r0:r0 + DH, hp, j, q0:],
                        start=True, stop=True)
                nc.scalar.activation(et[:, q0:TQ + w], ps[:, q0:TQ + w],
                                     ACTF.Exp, scale=float(SCALE))
                eview = et[:, q0:q0 + 2 * w].rearrange(
                    "p (j c) -> p j c", j=2)
                mv = [et[:, q0:TQ], et[:, TQ:TQ + w]]
            else:
                nc.tensor.matmul(ps[:, 0:2 * w],
                                 kT_t[:, cc, kc * 128:(kc + 1) * 128],
                                 qp_t[:, hp, :, q0:],
                                 start=True, stop=True)
                nc.scalar.activation(et[:, 0:2 * w], ps[:, 0:2 * w],
                                     ACTF.Exp, scale=float(SCALE))
                eview = et[:, 0:2 * w].rearrange(
                    "p (j c) -> p j c", j=2)
                mv = [et[:, j * w:(j + 1) * w] for j in range(2)]
            nc.vector.tensor_mul(eview[:, :, 0:32], eview[:, :, 0:32],
                                 msk_sb[:])
            return mv

        def _ctx_chunk(hp, kc, ets, ctx_live):
            q0 = 32 * kc
            for j in range(2):
                nc.tensor.matmul(ctx_live[j][:, q0:],
                                 v_t[:, kc, 2 * hp + j, :], ets[kc][j],
                                 start=(kc == 0), stop=(kc == KCN - 1))

        def _norm_ops(hp, ctx_live):
            ops = []
            for j in range(2):
                def _one(hp=hp, j=j, ctx_ps=ctx_live[j]):
                    cc, ro = hp, j * DH
                    # custom DVE/gpsimd ops need partition-0 operands
                    dnr = apool.tile([1, TQ], F32, name="dnr", tag="dnr",
                                     bufs=1)
                    nc.vector.tensor_copy(dnr[:], ctx_ps[DH:DH + 1, :])
                    rcp = apool.tile([1, TQ], F32, name="rcp", tag="rcp",
                                     bufs=1)
                    nc.vector.reciprocal_approx_fast(rcp[:], dnr[:])
                    bc = apool.tile([128, TQ], F32, name="bc", tag="bc",
                                    bufs=2)
                    nc.gpsimd.partition_broadcast(bc[:], rcp[:], channels=128)
                    # fused evict+normalize: (ctx * 1.0) * (1/denom)
                    nc.vector.scalar_tensor_tensor(
                        ctxT_t[ro:ro + DH, cc, :], ctx_ps[0:DH, :], 1.0,
                        bc[ro:ro + DH, :],
                        mybir.AluOpType.mult, mybir.AluOpType.mult)
                ops.append(_one)
            return ops

        # per pair: the 16 64-mode score matmuls run first (one PE-mode
        # switch), then ctx chunks interleave with the packed kc>=8 scores
        # (all full-array) so the PE never waits on the exp stream
        pending = []
        for hp in range(HP):
            # spread the 9.4MB W1/W2 prefetch across the pair loop so the
            # HBM burst doesn't collide with attention (and, across cores,
            # with itself)
            if hp < 3:
                nc.sync.dma_start(out=w1_t[:, 2 * hp:2 * hp + 2, :],
                                  in_=d["w1"][:, 2 * hp:2 * hp + 2, :])
            else:
                g = hp - 3
                nc.sync.dma_start(out=w2_t[:, 8 * g:8 * g + 8, :],
                                  in_=d["w2"][:, 8 * g:8 * g + 8, :])
            ctx_live = [pctx.tile([DH + 1, TQ], F32, name=f"ctx{j}",
                                  tag=f"ctx{j}") for j in range(2)]
            ets = {}
            if pending:
                pending.pop(0)()
            for kc in range(8):
                ets[kc] = _score_chunk(hp, kc)
            if pending:
                pending.pop(0)()
            # two ctx chunks per packed-score slot: the score stream (and
            # its exps) then extends to the end of the pair, so ACT never
            # idles through a ctx-only tail
            for i in range(8):
                ets[8 + i] = _score_chunk(hp, 8 + i)
                _ctx_chunk(hp, 2 * i, ets, ctx_live)
                _ctx_chunk(hp, 2 * i + 1, ets, ctx_live)
            pending.extend(_norm_ops(hp, ctx_live))
        for op in pending:
            op()

    kv_pool.release()
    qp_pool.release()

    # ==================== phase C: Wo + residual + LN1 ==================
    hT_holder = {}
    with tc.tile_pool(name="cpool", bufs=2, side="right") as cpool, \
         tc.tile_pool(name="r1pool", bufs=1, side="right") as r1pool:
        r1_t = r1pool.tile([128, CC, TQ], F32R, name="r1_t")
        with tc.tile_pool(name="pao", bufs=2, space="PSUM") as pao, \
             tc.tile_pool(name="pst", bufs=2, space="PSUM") as pst:
            ps_sum = pst.tile([1, TQ], F32, name="ps_sum", tag="st")
            ps_sq = pst.tile([1, TQ], F32, name="ps_sq", tag="st")
            sqs = []
            for mc in range(CC):
                ps = pao.tile([128, TQ], F32, name="ps_ao", tag="ao")
                for kc in range(CC):
                    nc.tensor.matmul(ps[:],
                                     wo_t[:, kc, mc * 128:(mc + 1) * 128],
                                     ctxT_t[:, kc, :],
                                     start=(kc == 0), stop=(kc == CC - 1))
                nc.vector.scalar_tensor_tensor(
                    r1_t[:, mc, :], ps[:], prm(mc, P_BO), xtq_t[:, mc, :],
                    mybir.AluOpType.add, mybir.AluOpType.add)
                sq = cpool.tile([128, TQ], F32R, name="sq", tag="sq", bufs=CC)
                nc.scalar.activation(sq[:], r1_t[:, mc, :], ACTF.Square)
                sqs.append(sq)
            # stats matmuls batched at the end: one PE mode-switch region
            for mc in range(CC):
                nc.tensor.matmul(ps_sum[:], ones_sb[:, 0:1], r1_t[:, mc, :],
                                 start=(mc == 0), stop=(mc == CC - 1))
            for mc in range(CC):
                nc.tensor.matmul(ps_sq[:], ones_sb[:, 0:1], sqs[mc][:],
                                 start=(mc == 0), stop=(mc == CC - 1))
            mean1, ex21 = _ln_stats(nc, cpool, ps_sum, ps_sq, "l1")
        hT_pool = tc.alloc_tile_pool(name="hTp", bufs=1, side="left")
        hTf_t = hT_pool.tile([128, CC, TQ], F32R, name="hTf_t")
        hTb_t = hT_pool.tile([128, CC, TQ], BF16, name="hTb_t")
        hT_holder["pool"] = hT_pool
        _ln_apply(nc, cpool, ones_sb, mean1, ex21,
                  lambda cc: r1_t[:, cc, :],
                  lambda cc: hTf_t[:, cc, :],
                  [prm(cc, P_L1S) for cc in range(CC)],
                  [prm(cc, P_L1B) for cc in range(CC)], "l1")
        for cc in range(CC):
            nc.vector.tensor_copy(hTb_t[:, cc, :], hTf_t[:, cc, :])

    xtq_pool.release()
    ctxT_pool.release()

    # ==================== phase D: MLP + residual + LN2 =================
    with tc.tile_pool(name="dpool", bufs=3, side="right") as dpool, \
         tc.tile_pool(name="r2pool", bufs=1, side="right") as r2pool:
        r2_t = r2pool.tile([128, CC, TQ], F32R, name="r2_t")
        with tc.tile_pool(name="pfc2", bufs=1, space="PSUM") as pfc2:
            ps_m = [pfc2.tile([128, TQ], F32, name=f"ps_m{mc}", tag=f"m{mc}")
                    for mc in range(CC)]
            with tc.tile_pool(name="pfc1", bufs=2, space="PSUM") as pfc1:
                for ic in range(ICN):
                    ps1 = pfc1.tile([128, TQ], F32, name="ps1", tag="f1")
                    for kc in range(CC):
                        nc.tensor.matmul(
                            ps1[:], w1_t[:, kc, ic * 128:(ic + 1) * 128],
                            hTb_t[:, kc, :],
                            start=(kc == 0), stop=(kc == CC - 1))
                    g = dpool.tile([128, TQ], BF16, name="g", tag="g")
                    nc.scalar.activation(g[:], ps1[:], ACTF.Gelu_apprx_tanh,
                                         bias=b1p_sb[:, ic].unsqueeze(-1))
                    for mc in range(CC):
                        nc.tensor.matmul(ps_m[mc][:],
                                         w2_t[:, ic, mc * 128:(mc + 1) * 128],
                                         g[:], start=(ic == 0),
                                         stop=(ic == ICN - 1))
            with tc.tile_pool(name="pst2", bufs=2, space="PSUM") as pst2:
                ps_sum2 = pst2.tile([1, TQ], F32, name="ps_sum2", tag="st")
                ps_sq2 = pst2.tile([1, TQ], F32, name="ps_sq2", tag="st")
                sqs2 = []
                for mc in range(CC):
                    nc.vector.scalar_tensor_tensor(
                        r2_t[:, mc, :], ps_m[mc][:], prm(mc, P_B2),
                        hTf_t[:, mc, :], mybir.AluOpType.add,
                        mybir.AluOpType.add)
                    sq = dpool.tile([128, TQ], F32R, name="sq2", tag="sq",
                                    bufs=CC)
                    nc.scalar.activation(sq[:], r2_t[:, mc, :], ACTF.Square)
                    sqs2.append(sq)
                for mc in range(CC):
                    nc.tensor.matmul(ps_sum2[:], ones_sb[:, 0:1],
                                     r2_t[:, mc, :],
                                     start=(mc == 0), stop=(mc == CC - 1))
                for mc in range(CC):
                    nc.tensor.matmul(ps_sq2[:], ones_sb[:, 0:1], sqs2[mc][:],
                                     start=(mc == 0), stop=(mc == CC - 1))
                mean2, ex22 = _ln_stats(nc, dpool, ps_sum2, ps_sq2, "l2")
        hT_holder["pool"].release()
        _ln_apply(nc, dpool, ones_sb, mean2, ex22,
                  lambda cc: r2_t[:, cc, :],
                  lambda cc: dpool.tile([128, TQ], F32, name="o",
                                        tag="o", bufs=2)[:],
                  [prm(cc, P_L2S) for cc in range(CC)],
                  [prm(cc, P_L2B) for cc in range(CC)], "l2",
                  post_f=lambda cc, ap: nc.sync.dma_start(
                      out=d["outT"][:, cc, :], in_=ap))

    wmlp_pool.release()
    wo_pool.release()
    const.release()


def _ln_stats(nc, pool, ps_sum, ps_sq, tagp):
    n = float(C)
    mean = pool.tile([1, TQ], F32, name="ln_mean", tag=tagp + "mean")
    nc.scalar.activation(mean[:], ps_sum[:], ACTF.Copy, scale=1.0 / n)
    ex2 = pool.tile([1, TQ], F32, name="ln_ex2", tag=tagp + "ex2")
    nc.scalar.activation(ex2[:], ps_sq[:], ACTF.Copy, scale=1.0 / n)
    return mean, ex2


def _ln_apply(nc, pool, ones_sb, mean, ex2, in_f, out_f,
              scales, biases, tagp, post_f=None):
    """out = (in - mean)/sqrt(var_unbiased + eps) * s + b, stats over C."""
    n = float(C)
    m2 = pool.tile([1, TQ], F32, name="ln_m2", tag=tagp + "m2", bufs=1)
    nc.vector.tensor_mul(m2[:], mean[:], mean[:])
    dv = pool.tile([1, TQ], F32, name="ln_d", tag=tagp + "d", bufs=1)
    nc.vector.tensor_sub(dv[:], ex2[:], m2[:])
    eps_sb = pool.tile([1, 1], F32, name="ln_eps", tag=tagp + "eps", bufs=1)
    nc.vector.memset(eps_sb[:], float(EPS))
    std = pool.tile([1, TQ], F32, name="ln_std", tag=tagp + "std", bufs=1)
    nc.scalar.activation(std[:], dv[:], ACTF.Sqrt,
                         scale=n / (n - 1.0), bias=eps_sb[:])
    istd = pool.tile([1, TQ], F32, name="ln_istd", tag=tagp + "istd", bufs=1)
    nc.vector.reciprocal_approx_fast(istd[:], std[:])
    # broadcast mean and istd across partitions on the (idle) gpsimd engine
    mb = pool.tile([128, TQ], F32, name="ln_mb", tag=tagp + "mb", bufs=1)
    nc.gpsimd.partition_broadcast(mb[:], mean[:], channels=128)
    ib = pool.tile([128, TQ], F32, name="ln_ib", tag=tagp + "ib", bufs=1)
    nc.gpsimd.partition_broadcast(ib[:], istd[:], channels=128)
    for cc in range(CC):
        t1 = pool.tile([128, TQ], F32, name="ln_t1", tag=tagp + "t1", bufs=2)
        nc.vector.tensor_sub(t1[:], in_f(cc), mb[:])
        t2 = pool.tile([128, TQ], F32, name="ln_t2", tag=tagp + "t2", bufs=2)
        nc.vector.tensor_mul(t2[:], t1[:], ib[:])
        out_ap = out_f(cc)
        nc.vector.tensor_scalar(out_ap, t2[:], scales[cc], biases[cc],
                                mybir.AluOpType.mult, mybir.AluOpType.add)
        if post_f is not None:
            post_f(cc, out_ap)


_NC = None


def _get_nc():
    global _NC
    if _NC is None:
        _NC = _build_nc()
    return _NC


def _prep_inmaps(x, Wq, bq, Wk, bk, Wv, bv, Wo, bo, ln1_s, ln1_b,
                 W1, b1, W2, b2, ln2_s, ln2_b):
    f32 = np.float32
    bf16 = ml_dtypes.bfloat16

    def wpack(w, nch, width):
        # [nch*128, width] -> [128, nch, width]
        return np.ascontiguousarray(
            np.asarray(w, f32).reshape(nch, 128, width).transpose(1, 0, 2)
        ).astype(bf16)

    wq = wpack(Wq, CC, C)
    wk = wpack(Wk, CC, C)
    wv = wpack(Wv, CC, C)
    wo = wpack(Wo, CC, C)
    w1 = wpack(W1, CC, IN)
    w2 = wpack(W2, ICN, C)
    prk = np.zeros((128, CC, 8), f32)
    for pi, arr in ((P_BQ, bq), (P_BK, bk), (P_BO, bo), (P_B2, b2),
                    (P_L1S, ln1_s), (P_L1B, ln1_b), (P_L2S, ln2_s),
                    (P_L2B, ln2_b)):
        prk[:, :, pi] = np.asarray(arr, f32).reshape(CC, 128).T
    b1p = np.ascontiguousarray(np.asarray(b1, f32).reshape(ICN, 128).T)
    bvb = np.broadcast_to(np.asarray(bv, f32)[None, :], (128, C)).copy()
    ones = np.ones((128, 128), f32)

    xT = [np.ascontiguousarray(np.asarray(x)[b].T, dtype=f32)
          for b in range(B)]
    kk = np.arange(128)[:, None]
    qq = np.arange(32)[None, :]
    in_maps = []
    for c in range(8):
        b, p = c // 4, c % 4
        xtq = np.ascontiguousarray(
            xT[b][:, p::4].reshape(CC, 128, TQ).transpose(1, 0, 2))
        xtb = np.ascontiguousarray(
            xT[b].reshape(CC, 128, T).transpose(1, 0, 2)).astype(bf16)
        msk = np.zeros((128, 2, 32), bf16)
        msk[:, 0, :] = (kk <= 4 * qq + p).astype(bf16)
        msk[:, 1, :] = msk[:, 0, :]
        in_maps.append({
            "xtq": xtq, "xtqb": xtq.astype(bf16), "xtb": xtb,
            "wq": wq, "wk": wk, "wv": wv, "wo": wo, "w1": w1, "w2": w2,
            "prk": prk, "b1p": b1p, "bvb": bvb, "msk": msk, "ones": ones,
        })
    return in_maps


def _run(in_maps, trace=False, **kw):
    nc = _get_nc()
    return run_bass_kernel_spmd(nc, in_maps, list(range(8)), trace=trace, **kw)


def kernel(**inputs):
    in_maps = _prep_inmaps(**inputs)
    res = _run(in_maps)
    out = np.empty((B, T, C), np.float32)
    for c in range(8):
        b, p = c // 4, c % 4
        o = res.results[c]["outT"]  # [128, CC, TQ]
        out[b, p::4, :] = o.transpose(1, 0, 2).reshape(C, TQ).T
    return out


# revision 52
# speedup vs baseline: 1.0023x; 1.0023x over previous
"""Trainium2 Bass kernel for one transformer block (B=2, T=2048, C=768, H=12,
inner=3072, fp32 io, causal, post-norm residual).

Sharding: 8 cores, token-interleaved. Core c handles batch c//4, tokens
p::4 (p = c%4) of that batch - every core runs the IDENTICAL program
(SPMD), causality is data-driven via a per-core diagonal-sliver mask.

Design (341us f32r baseline -> 296us):
 - all matmul operands bf16 (weights, x, q/k/v, ctx, h, gelu out): halves
   HBM traffic, enables FWL fast weight load, no f32r 256-col floor.
   Residual stream (r1/r2) and LN math stay f32/f32r (rel err ~3e-3).
 - causality: score matmul for key chunk kc covers only queries >= 32*kc;
   only the 32-query diagonal sliver needs masking (mask is chunk-
   independent: key k_local visible iff k_local <= 4*q_local + p), so the
   DVE mask work is ~10x smaller than masking every score.
 - attention per head pair (pair hp shares kT chunk cc=hp): kc<8 scores
   use 64-row PE tiling (per-head array halves -> different PSUM banks);
   kc>=8 pack both heads' scores into one 512-f32 bank via the
   zero-padded-q trick. ctx chunks interleave with the kc>=8 scores so
   the PE rides the exp (ACT) stream without stalling; softmax
   denominators ride as a ones-column on V, reciprocals via the fast
   approx DVE op, partition-broadcast on the (pre-warmed) gpsimd engine.
 - Wo/W1/W2 stream from HBM during attention (SBUF freed by bf16);
   LN stats accumulate on the PE via ones-matmuls, batched to limit
   PE array-tiling mode switches.

Known pitfalls encoded here: one matmul output may not span PSUM banks;
two row-tiles must never write the same bank; custom DVE/gpsimd ops
require partition-0 operands; partition-shifted plain DVE ops need
32-aligned bases; gpsimd's first custom-op dispatch costs ~10us.
"""

import sys

if "/opt/trn_rl_repo" not in sys.path:
    sys.path.insert(0, "/opt/trn_rl_repo")

import numpy as np
import ml_dtypes

import concourse.bacc as bacc
import concourse.mybir as mybir
import concourse.tile as tile
from concourse.bass_utils import run_bass_kernel_spmd

F32 = mybir.dt.float32
F32R = mybir.dt.float32r
BF16 = mybir.dt.bfloat16
ACTF = mybir.ActivationFunctionType

B, T, C = 2, 2048, 768
H, DH = 12, 64
IN = 3072
CC = C // 128           # 6 channel chunks
TQ = 512                # tokens per core
KCN = T // 128          # 16 key chunks
ICN = IN // 128         # 24 inner chunks
HP = H // 2             # 6 head pairs
EPS = 1e-4
SCALE = 1.0 / np.sqrt(DH)

# param pack order in "prk" [128, CC, 8]
P_BQ, P_BK, P_BO, P_B2, P_L1S, P_L1B, P_L2S, P_L2B = range(8)


def _build_nc():
    nc = bacc.Bacc("TRN2", target_bir_lowering=False, debug=False,
                   enable_asserts=False, num_devices=8)
    d = {}
    d["xtq"] = nc.dram_tensor("xtq", [128, CC, TQ], F32, kind="ExternalInput").ap()
    d["xtqb"] = nc.dram_tensor("xtqb", [128, CC, TQ], BF16, kind="ExternalInput").ap()
    d["xtb"] = nc.dram_tensor("xtb", [128, CC, T], BF16, kind="ExternalInput").ap()
    for w, sh in (("wq", [128, CC, C]), ("wk", [128, CC, C]),
                  ("wv", [128, CC, C]), ("wo", [128, CC, C]),
                  ("w1", [128, CC, IN]), ("w2", [128, ICN, C])):
        d[w] = nc.dram_tensor(w, sh, BF16, kind="ExternalInput").ap()
    d["prk"] = nc.dram_tensor("prk", [128, CC, 8], F32, kind="ExternalInput").ap()
    d["b1p"] = nc.dram_tensor("b1p", [128, ICN], F32, kind="ExternalInput").ap()
    d["bvb"] = nc.dram_tensor("bvb", [128, C], F32, kind="ExternalInput").ap()
    d["msk"] = nc.dram_tensor("msk", [128, 2, 32], BF16, kind="ExternalInput").ap()
    d["ones"] = nc.dram_tensor("ones", [128, 128], F32R, kind="ExternalInput").ap()
    d["outT"] = nc.dram_tensor("outT", [128, CC, TQ], F32, kind="ExternalOutput").ap()

    with tile.TileContext(nc) as tc:
        _emit(nc, tc, d)
    nc.finalize()
    return nc


def _emit(nc, tc, d):
    # ---- persistent constants ------------------------------------------
    const = tc.alloc_tile_pool(name="const", bufs=1, side="left")
    ones_sb = const.tile([128, 128], F32R, name="ones_sb")
    prk_sb = const.tile([128, CC, 8], F32, name="prk_sb")
    b1p_sb = const.tile([128, ICN], F32, name="b1p_sb")
    bvb_sb = const.tile([128, C], F32, name="bvb_sb")
    msk_sb = const.tile([128, 2, 32], BF16, name="msk_sb")

    def prm(cc, pi):
        return prk_sb[:, cc, pi].unsqueeze(-1)  # [128,1]

    nc.sync.dma_start(out=prk_sb[:], in_=d["prk"][:])
    nc.sync.dma_start(out=msk_sb[:], in_=d["msk"][:])

    wo_pool = tc.alloc_tile_pool(name="wop", bufs=1, side="left")
    wo_t = wo_pool.tile([128, CC, C], BF16, name="wo_t")

    # right-side stack (LIFO): ctxT/xtq (A-C), qp/kv (A-B), xtqb/xtb (A)
    ctxT_pool = tc.alloc_tile_pool(name="ctxTp", bufs=1, side="right")
    ctxT_t = ctxT_pool.tile([128, CC, TQ], BF16, name="ctxT_t")
    xtq_pool = tc.alloc_tile_pool(name="xtq", bufs=1, side="right")
    xtq_t = xtq_pool.tile([128, CC, TQ], F32, name="xtq_t")
    qp_pool = tc.alloc_tile_pool(name="qp", bufs=1, side="right")
    qp_t = qp_pool.tile([128, CC, 2, TQ], BF16, name="qp_t")
    kv_pool = tc.alloc_tile_pool(name="kvp", bufs=1, side="right")
    kT_t = kv_pool.tile([128, CC, T], BF16, name="kT_t")
    v_t = kv_pool.tile([128, KCN, H, DH + 1], BF16, name="v_t")
    xtqb_pool = tc.alloc_tile_pool(name="xtqb", bufs=1, side="right")
    xtqb_t = xtqb_pool.tile([128, CC, TQ], BF16, name="xtqb_t")
    xtb_pool = tc.alloc_tile_pool(name="xtb", bufs=1, side="right")
    xtb_t = xtb_pool.tile([128, CC, T], BF16, name="xtb_t")

    # ==================== phase A: QKV ==================================
    with tc.tile_pool(name="wqkv", bufs=1, side="right") as wqkv, \
         tc.tile_pool(name="pqkv", bufs=2, space="PSUM") as pqkv:
        wq_t = wqkv.tile([128, CC, C], BF16, name="wq_t", tag="wq")
        wk_t = wqkv.tile([128, CC, C], BF16, name="wk_t", tag="wk")
        wv_t = wqkv.tile([128, CC, C], BF16, name="wv_t", tag="wv")
        nc.sync.dma_start(out=xtqb_t[:], in_=d["xtqb"][:])
        for i in range(3):
            nc.sync.dma_start(out=wq_t[:, 2 * i:2 * i + 2, :],
                              in_=d["wq"][:, 2 * i:2 * i + 2, :])
        nc.sync.dma_start(out=bvb_sb[:], in_=d["bvb"][:])
        nc.sync.dma_start(out=wv_t[:], in_=d["wv"][:])
        for i in range(3):
            nc.sync.dma_start(out=xtb_t[:, 2 * i:2 * i + 2, :],
                              in_=d["xtb"][:, 2 * i:2 * i + 2, :])
        nc.sync.dma_start(out=wk_t[:], in_=d["wk"][:])
        nc.sync.dma_start(out=ones_sb[:], in_=d["ones"][:])
        nc.sync.dma_start(out=b1p_sb[:], in_=d["b1p"][:])
        nc.sync.dma_start(out=xtq_t[:], in_=d["xtq"][:])

        # zero q staging (evicts below overwrite the live rows)
        nc.vector.memset(qp_t[:], 0.0)

        # warm up the gpsimd custom-instruction library off the critical
        # path (first dispatch pays a ~10us library-load/warmup penalty)
        gwarm = wqkv.tile([128, 16], F32R, name="gwarm", tag="gw")
        nc.gpsimd.partition_broadcast(gwarm[:], ones_sb[0:1, 0:16],
                                      channels=128)

        # q^T: per output chunk mc, heads 2mc (rows 0:64) / 2mc+1 (64:128)
        for mc in range(CC):
            ps = pqkv.tile([128, TQ], F32, name="ps_q", tag="pk", bufs=3)
            for kc in range(CC):
                nc.tensor.matmul(ps[:], wq_t[:, kc, mc * 128:(mc + 1) * 128],
                                 xtqb_t[:, kc, :],
                                 start=(kc == 0), stop=(kc == CC - 1))
            nc.vector.tensor_scalar_add(
                qp_t[0:64, mc, 0, :], ps[0:64, :], prm(mc, P_BQ)[0:64, :])
            nc.vector.tensor_scalar_add(
                qp_t[64:128, mc, 1, :], ps[64:128, :], prm(mc, P_BQ)[64:128, :])

        # v rows (token-major) + ones column for softmax denominators;
        # before k so the heavy v evicts drain while k matmuls run
        nc.vector.memset(v_t[:, :, :, DH], 1.0)
        for tch in range(KCN):
            ps1 = pqkv.tile([128, 512], F32, name="ps_v1", tag="pv1")
            ps2 = pqkv.tile([128, 256], F32, name="ps_v2", tag="pv2")
            for kc in range(CC):
                xsl = xtb_t[:, kc, tch * 128:(tch + 1) * 128]
                nc.tensor.matmul(ps1[:], xsl, wv_t[:, kc, 0:512],
                                 start=(kc == 0), stop=(kc == CC - 1))
                nc.tensor.matmul(ps2[:], xsl, wv_t[:, kc, 512:C],
                                 start=(kc == 0), stop=(kc == CC - 1))
            nc.vector.tensor_add(
                v_t[:, tch, 0:8, 0:DH],
                ps1[:].rearrange("p (h d) -> p h d", d=DH),
                bvb_sb[:, 0:512].rearrange("p (h d) -> p h d", d=DH))
            nc.vector.tensor_add(
                v_t[:, tch, 8:H, 0:DH],
                ps2[:].rearrange("p (h d) -> p h d", d=DH),
                bvb_sb[:, 512:C].rearrange("p (h d) -> p h d", d=DH))

        # k^T for the full 2048 keys, chunk mc ascending (attention pair 0
        # needs chunk 0 first)
        for mc in range(CC):
            for tb in range(4):
                ps = pqkv.tile([128, 512], F32, name="ps_k", tag="pk", bufs=3)
                for kc in range(CC):
                    nc.tensor.matmul(
                        ps[:], wk_t[:, kc, mc * 128:(mc + 1) * 128],
                        xtb_t[:, kc, tb * 512:(tb + 1) * 512],
                        start=(kc == 0), stop=(kc == CC - 1))
                nc.vector.tensor_scalar_add(
                    kT_t[:, mc, tb * 512:(tb + 1) * 512], ps[:],
                    prm(mc, P_BK))

    xtb_pool.release()
    xtqb_pool.release()

    # prefetch weights for later phases while attention runs (the pool
    # reuses the SBUF space xtb/xtqb just released)
    nc.sync.dma_start(out=wo_t[:], in_=d["wo"][:])
    wmlp_pool = tc.alloc_tile_pool(name="wmlp", bufs=1, side="left")
    w1_t = wmlp_pool.tile([128, CC, IN], BF16, name="w1_t")
    w2_t = wmlp_pool.tile([128, ICN, C], BF16, name="w2_t")

    # ==================== phase B: attention ============================
    with tc.tile_pool(name="attnp", bufs=4, side="right") as apool, \
         tc.tile_pool(name="psc", bufs=3, space="PSUM") as psc, \
         tc.tile_pool(name="pctx", bufs=1, space="PSUM") as pctx:

        def _score_chunk(hp, kc):
            """Scores + exp + mask for one key chunk. kc<8 uses 64-row PE
            tiling (both heads on independent array halves, different PSUM
            banks); kc>=8 one packed full-array matmul."""
            cc = hp
            q0 = 32 * kc
            w = TQ - q0
            ps = psc.tile([128, 2 * TQ], F32, name="ps_s", tag="s")
            et = apool.tile([128, 2 * TQ], BF16, name="et", tag="e",
                            bufs=KCN + 2)
            if kc < 8:
                for j in range(2):
                    r0, off = j * DH, q0 if j == 0 else TQ
                    nc.tensor.matmul(
                        ps[:, off:off + w],
                        kT_t[r0:r0 + DH, cc, kc * 128:(kc + 1) * 128],
                        qp_t[r0:r0 + DH, hp, j, q0:],
                        start=True, stop=True)
                nc.scalar.activation(et[:, q0:TQ + w], ps[:, q0:TQ + w],
                                     ACTF.Exp, scale=float(SCALE))
                eview = et[:, q0:q0 + 2 * w].rearrange(
                    "p (j c) -> p j c", j=2)
                mv = [et[:, q0:TQ], et[:, TQ:TQ + w]]
            else:
                nc.tensor.matmul(ps[:, 0:2 * w],
                                 kT_t[:, cc, kc * 128:(kc + 1) * 128],
                                 qp_t[:, hp, :, q0:],
                                 start=True, stop=True)
                nc.scalar.activation(et[:, 0:2 * w], ps[:, 0:2 * w],
                                     ACTF.Exp, scale=float(SCALE))
                eview = et[:, 0:2 * w].rearrange(
                    "p (j c) -> p j c", j=2)
                mv = [et[:, j * w:(j + 1) * w] for j in range(2)]
            nc.vector.tensor_mul(eview[:, :, 0:32], eview[:, :, 0:32],
                                 msk_sb[:])
            return mv

        def _ctx_chunk(hp, kc, ets, ctx_live):
            q0 = 32 * kc
            for j in range(2):
                nc.tensor.matmul(ctx_live[j][:, q0:],
                                 v_t[:, kc, 2 * hp + j, :], ets[kc][j],
                                 start=(kc == 0), stop=(kc == KCN - 1))

        def _norm_ops(hp, ctx_live):
            ops = []
            for j in range(2):
                def _one(hp=hp, j=j, ctx_ps=ctx_live[j]):
                    cc, ro = hp, j * DH
                    # custom DVE/gpsimd ops need partition-0 operands
                    dnr = apool.tile([1, TQ], F32, name="dnr", tag="dnr",
                                     bufs=1)
                    nc.vector.tensor_copy(dnr[:], ctx_ps[DH:DH + 1, :])
                    rcp = apool.tile([1, TQ], F32, name="rcp", tag="rcp",
                                     bufs=1)
                    nc.vector.reciprocal_approx_fast(rcp[:], dnr[:])
                    bc = apool.tile([128, TQ], F32, name="bc", tag="bc",
                                    bufs=2)
                    nc.gpsimd.partition_broadcast(bc[:], rcp[:], channels=128)
                    # fused evict+normalize: (ctx * 1.0) * (1/denom)
                    nc.vector.scalar_tensor_tensor(
                        ctxT_t[ro:ro + DH, cc, :], ctx_ps[0:DH, :], 1.0,
                        bc[ro:ro + DH, :],
                        mybir.AluOpType.mult, mybir.AluOpType.mult)
                ops.append(_one)
            return ops

        # per pair: the 16 64-mode score matmuls run first (one PE-mode
        # switch), then ctx chunks interleave with the packed kc>=8 scores
        # (all full-array) so the PE never waits on the exp stream
        pending = []
        for hp in range(HP):
            # spread the 9.4MB W1/W2 prefetch across the pair loop so the
            # HBM burst doesn't collide with attention (and, across cores,
            # with itself)
            if hp < 3:
                nc.sync.dma_start(out=w1_t[:, 2 * hp:2 * hp + 2, :],
                                  in_=d["w1"][:, 2 * hp:2 * hp + 2, :])
            else:
                g = hp - 3
                nc.sync.dma_start(out=w2_t[:, 8 * g:8 * g + 8, :],
                                  in_=d["w2"][:, 8 * g:8 * g + 8, :])
            ctx_live = [pctx.tile([DH + 1, TQ], F32, name=f"ctx{j}",
                                  tag=f"ctx{j}") for j in range(2)]
            ets = {}
            if pending:
                pending.pop(0)()
            for kc in range(8):
                ets[kc] = _score_chunk(hp, kc)
            if pending:
                pending.pop(0)()
            for i in range(8):
                ets[8 + i] = _score_chunk(hp, 8 + i)
                _ctx_chunk(hp, i, ets, ctx_live)
            for kc in range(8, KCN):
                _ctx_chunk(hp, kc, ets, ctx_live)
            pending.extend(_norm_ops(hp, ctx_live))
        for op in pending:
            op()

    kv_pool.release()
    qp_pool.release()

    # ==================== phase C: Wo + residual + LN1 ==================
    hT_holder = {}
    with tc.tile_pool(name="cpool", bufs=2, side="right") as cpool, \
         tc.tile_pool(name="r1pool", bufs=1, side="right") as r1pool:
        r1_t = r1pool.tile([128, CC, TQ], F32R, name="r1_t")
        with tc.tile_pool(name="pao", bufs=2, space="PSUM") as pao, \
             tc.tile_pool(name="pst", bufs=2, space="PSUM") as pst:
            ps_sum = pst.tile([1, TQ], F32, name="ps_sum", tag="st")
            ps_sq = pst.tile([1, TQ], F32, name="ps_sq", tag="st")
            sqs = []
            for mc in range(CC):
                ps = pao.tile([128, TQ], F32, name="ps_ao", tag="ao")
                for kc in range(CC):
                    nc.tensor.matmul(ps[:],
                                     wo_t[:, kc, mc * 128:(mc + 1) * 128],
                                     ctxT_t[:, kc, :],
                                     start=(kc == 0), stop=(kc == CC - 1))
                nc.vector.scalar_tensor_tensor(
                    r1_t[:, mc, :], ps[:], prm(mc, P_BO), xtq_t[:, mc, :],
                    mybir.AluOpType.add, mybir.AluOpType.add)
                sq = cpool.tile([128, TQ], F32R, name="sq", tag="sq", bufs=CC)
                nc.scalar.activation(sq[:], r1_t[:, mc, :], ACTF.Square)
                sqs.append(sq)
            # stats matmuls batched at the end: one PE mode-switch region
            for mc in range(CC):
                nc.tensor.matmul(ps_sum[:], ones_sb[:, 0:1], r1_t[:, mc, :],
                                 start=(mc == 0), stop=(mc == CC - 1))
            for mc in range(CC):
                nc.tensor.matmul(ps_sq[:], ones_sb[:, 0:1], sqs[mc][:],
                                 start=(mc == 0), stop=(mc == CC - 1))
            mean1, ex21 = _ln_stats(nc, cpool, ps_sum, ps_sq, "l1")
        hT_pool = tc.alloc_tile_pool(name="hTp", bufs=1, side="left")
        hTf_t = hT_pool.tile([128, CC, TQ], F32R, name="hTf_t")
        hTb_t = hT_pool.tile([128, CC, TQ], BF16, name="hTb_t")
        hT_holder["pool"] = hT_pool
        _ln_apply(nc, cpool, ones_sb, mean1, ex21,
                  lambda cc: r1_t[:, cc, :],
                  lambda cc: hTf_t[:, cc, :],
                  [prm(cc, P_L1S) for cc in range(CC)],
                  [prm(cc, P_L1B) for cc in range(CC)], "l1")
        for cc in range(CC):
            nc.vector.tensor_copy(hTb_t[:, cc, :], hTf_t[:, cc, :])

    xtq_pool.release()
    ctxT_pool.release()

    # ==================== phase D: MLP + residual + LN2 =================
    with tc.tile_pool(name="dpool", bufs=3, side="right") as dpool, \
         tc.tile_pool(name="r2pool", bufs=1, side="right") as r2pool:
        r2_t = r2pool.tile([128, CC, TQ], F32R, name="r2_t")
        with tc.tile_pool(name="pfc2", bufs=1, space="PSUM") as pfc2:
            ps_m = [pfc2.tile([128, TQ], F32, name=f"ps_m{mc}", tag=f"m{mc}")
                    for mc in range(CC)]
            with tc.tile_pool(name="pfc1", bufs=2, space="PSUM") as pfc1:
                for ic in range(ICN):
                    ps1 = pfc1.tile([128, TQ], F32, name="ps1", tag="f1")
                    for kc in range(CC):
                        nc.tensor.matmul(
                            ps1[:], w1_t[:, kc, ic * 128:(ic + 1) * 128],
                            hTb_t[:, kc, :],
                            start=(kc == 0), stop=(kc == CC - 1))
                    g = dpool.tile([128, TQ], BF16, name="g", tag="g")
                    nc.scalar.activation(g[:], ps1[:], ACTF.Gelu_apprx_tanh,
                                         bias=b1p_sb[:, ic].unsqueeze(-1))
                    for mc in range(CC):
                        nc.tensor.matmul(ps_m[mc][:],
                                         w2_t[:, ic, mc * 128:(mc + 1) * 128],
                                         g[:], start=(ic == 0),
                                         stop=(ic == ICN - 1))
            with tc.tile_pool(name="pst2", bufs=2, space="PSUM") as pst2:
                ps_sum2 = pst2.tile([1, TQ], F32, name="ps_sum2", tag="st")
                ps_sq2 = pst2.tile([1, TQ], F32, name="ps_sq2", tag="st")
                sqs2 = []
                for mc in range(CC):
                    nc.vector.scalar_tensor_tensor(
                        r2_t[:, mc, :], ps_m[mc][:], prm(mc, P_B2),
                        hTf_t[:, mc, :], mybir.AluOpType.add,
                        mybir.AluOpType.add)
                    sq = dpool.tile([128, TQ], F32R, name="sq2", tag="sq",
                                    bufs=CC)
                    nc.scalar.activation(sq[:], r2_t[:, mc, :], ACTF.Square)
                    sqs2.append(sq)
                for mc in range(CC):
                    nc.tensor.matmul(ps_sum2[:], ones_sb[:, 0:1],
                                     r2_t[:, mc, :],
                                     start=(mc == 0), stop=(mc == CC - 1))
                for mc in range(CC):
                    nc.tensor.matmul(ps_sq2[:], ones_sb[:, 0:1], sqs2[mc][:],
                                     start=(mc == 0), stop=(mc == CC - 1))
                mean2, ex22 = _ln_stats(nc, dpool, ps_sum2, ps_sq2, "l2")
        hT_holder["pool"].release()
        _ln_apply(nc, dpool, ones_sb, mean2, ex22,
                  lambda cc: r2_t[:, cc, :],
                  lambda cc: dpool.tile([128, TQ], F32, name="o",
                                        tag="o", bufs=2)[:],
                  [prm(cc, P_L2S) for cc in range(CC)],
                  [prm(cc, P_L2B) for cc in range(CC)], "l2",
                  post_f=lambda cc, ap: nc.sync.dma_start(
                      out=d["outT"][:, cc, :], in_=ap))

    wmlp_pool.release()
    wo_pool.release()
    const.release()


def _ln_stats(nc, pool, ps_sum, ps_sq, tagp):
    n = float(C)
    mean = pool.tile([1, TQ], F32, name="ln_mean", tag=tagp + "mean")
    nc.scalar.activation(mean[:], ps_sum[:], ACTF.Copy, scale=1.0 / n)
    ex2 = pool.tile([1, TQ], F32, name="ln_ex2", tag=tagp + "ex2")
    nc.scalar.activation(ex2[:], ps_sq[:], ACTF.Copy, scale=1.0 / n)
    return mean, ex2


def _ln_apply(nc, pool, ones_sb, mean, ex2, in_f, out_f,
              scales, biases, tagp, post_f=None):
    """out = (in - mean)/sqrt(var_unbiased + eps) * s + b, stats over C."""
    n = float(C)
    m2 = pool.tile([1, TQ], F32, name="ln_m2", tag=tagp + "m2", bufs=1)
    nc.vector.tensor_mul(m2[:], mean[:], mean[:])
    dv = pool.tile([1, TQ], F32, name="ln_d", tag=tagp + "d", bufs=1)
    nc.vector.tensor_sub(dv[:], ex2[:], m2[:])
    eps_sb = pool.tile([1, 1], F32, name="ln_eps", tag=tagp + "eps", bufs=1)
    nc.vector.memset(eps_sb[:], float(EPS))
    std = pool.tile([1, TQ], F32, name="ln_std", tag=tagp + "std", bufs=1)
    nc.scalar.activation(std[:], dv[:], ACTF.Sqrt,
                         scale=n / (n - 1.0), bias=eps_sb[:])
    istd = pool.tile([1, TQ], F32, name="ln_istd", tag=tagp + "istd", bufs=1)
    nc.vector.reciprocal_approx_fast(istd[:], std[:])
    # broadcast mean and istd across partitions on the (idle) gpsimd engine
    mb = pool.tile([128, TQ], F32, name="ln_mb", tag=tagp + "mb", bufs=1)
    nc.gpsimd.partition_broadcast(mb[:], mean[:], channels=128)
    ib = pool.tile([128, TQ], F32, name="ln_ib", tag=tagp + "ib", bufs=1)
    nc.gpsimd.partition_broadcast(ib[:], istd[:], channels=128)
    for cc in range(CC):
        t1 = pool.tile([128, TQ], F32, name="ln_t1", tag=tagp + "t1", bufs=2)
        nc.vector.tensor_sub(t1[:], in_f(cc), mb[:])
        t2 = pool.tile([128, TQ], F32, name="ln_t2", tag=tagp + "t2", bufs=2)
        nc.vector.tensor_mul(t2[:], t1[:], ib[:])
        out_ap = out_f(cc)
        nc.vector.tensor_scalar(out_ap, t2[:], scales[cc], biases[cc],
                                mybir.AluOpType.mult, mybir.AluOpType.add)
        if post_f is not None:
            post_f(cc, out_ap)


_NC = None


def _get_nc():
    global _NC
    if _NC is None:
        _NC = _build_nc()
    return _NC


def _prep_inmaps(x, Wq, bq, Wk, bk, Wv, bv, Wo, bo, ln1_s, ln1_b,
                 W1, b1, W2, b2, ln2_s, ln2_b):
    f32 = np.float32
    bf16 = ml_dtypes.bfloat16

    def wpack(w, nch, width):
        # [nch*128, width] -> [128, nch, width]
        return np.ascontiguousarray(
            np.asarray(w, f32).reshape(nch, 128, width).transpose(1, 0, 2)
        ).astype(bf16)

    wq = wpack(Wq, CC, C)
    wk = wpack(Wk, CC, C)
    wv = wpack(Wv, CC, C)
    wo = wpack(Wo, CC, C)
    w1 = wpack(W1, CC, IN)
    w2 = wpack(W2, ICN, C)
    prk = np.zeros((128, CC, 8), f32)
    for pi, arr in ((P_BQ, bq), (P_BK, bk), (P_BO, bo), (P_B2, b2),
                    (P_L1S, ln1_s), (P_L1B, ln1_b), (P_L2S, ln2_s),
                    (P_L2B, ln2_b)):
        prk[:, :, pi] = np.asarray(arr, f32).reshape(CC, 128).T
    b1p = np.ascontiguousarray(np.asarray(b1, f32).reshape(ICN, 128).T)
    bvb = np.broadcast_to(np.asarray(bv, f32)[None, :], (128, C)).copy()
    ones = np.ones((128, 128), f32)

    xT = [np.ascontiguousarray(np.asarray(x)[b].T, dtype=f32)
          for b in range(B)]
    kk = np.arange(128)[:, None]
    qq = np.arange(32)[None, :]
    in_maps = []
    for c in range(8):
        b, p = c // 4, c % 4
        xtq = np.ascontiguousarray(
            xT[b][:, p::4].reshape(CC, 128, TQ).transpose(1, 0, 2))
        xtb = np.ascontiguousarray(
            xT[b].reshape(CC, 128, T).transpose(1, 0, 2)).astype(bf16)
        msk = np.zeros((128, 2, 32), bf16)
        msk[:, 0, :] = (kk <= 4 * qq + p).astype(bf16)
        msk[:, 1, :] = msk[:, 0, :]
        in_maps.append({
            "xtq": xtq, "xtqb": xtq.astype(bf16), "xtb": xtb,
            "wq": wq, "wk": wk, "wv": wv, "wo": wo, "w1": w1, "w2": w2,
            "prk": prk, "b1p": b1p, "bvb": bvb, "msk": msk, "ones": ones,
        })
    return in_maps


def _run(in_maps, trace=False, **kw):
    nc = _get_nc()
    return run_bass_kernel_spmd(nc, in_maps, list(range(8)), trace=trace, **kw)


def kernel(**inputs):
    in_maps = _prep_inmaps(**inputs)
    res = _run(in_maps)
    out = np.empty((B, T, C), np.float32)
    for c in range(8):
        b, p = c // 4, c % 4
        o = res.results[c]["outT"]  # [128, CC, TQ]
        out[b, p::4, :] = o.transpose(1, 0, 2).reshape(C, TQ).T
    return out


# revision 53
# speedup vs baseline: 1.0128x; 1.0105x over previous
"""Trainium2 Bass kernel for one transformer block (B=2, T=2048, C=768, H=12,
inner=3072, fp32 io, causal, post-norm residual).

Sharding: 8 cores, token-interleaved. Core c handles batch c//4, tokens
p::4 (p = c%4) of that batch - every core runs the IDENTICAL program
(SPMD), causality is data-driven via a per-core diagonal-sliver mask.

Design (341us f32r baseline -> 296us):
 - all matmul operands bf16 (weights, x, q/k/v, ctx, h, gelu out): halves
   HBM traffic, enables FWL fast weight load, no f32r 256-col floor.
   Residual stream (r1/r2) and LN math stay f32/f32r (rel err ~3e-3).
 - causality: score matmul for key chunk kc covers only queries >= 32*kc;
   only the 32-query diagonal sliver needs masking (mask is chunk-
   independent: key k_local visible iff k_local <= 4*q_local + p), so the
   DVE mask work is ~10x smaller than masking every score.
 - attention per head pair (pair hp shares kT chunk cc=hp): kc<8 scores
   use 64-row PE tiling (per-head array halves -> different PSUM banks);
   kc>=8 pack both heads' scores into one 512-f32 bank via the
   zero-padded-q trick. ctx chunks interleave with the kc>=8 scores so
   the PE rides the exp (ACT) stream without stalling; softmax
   denominators ride as a ones-column on V, reciprocals via the fast
   approx DVE op, partition-broadcast on the (pre-warmed) gpsimd engine.
 - Wo/W1/W2 stream from HBM during attention (SBUF freed by bf16);
   LN stats accumulate on the PE via ones-matmuls, batched to limit
   PE array-tiling mode switches.

Known pitfalls encoded here: one matmul output may not span PSUM banks;
two row-tiles must never write the same bank; custom DVE/gpsimd ops
require partition-0 operands; partition-shifted plain DVE ops need
32-aligned bases; gpsimd's first custom-op dispatch costs ~10us.
"""

import sys

if "/opt/trn_rl_repo" not in sys.path:
    sys.path.insert(0, "/opt/trn_rl_repo")

import numpy as np
import ml_dtypes

import concourse.bacc as bacc
import concourse.mybir as mybir
import concourse.tile as tile
from concourse.bass_utils import run_bass_kernel_spmd

F32 = mybir.dt.float32
F32R = mybir.dt.float32r
BF16 = mybir.dt.bfloat16
ACTF = mybir.ActivationFunctionType

B, T, C = 2, 2048, 768
H, DH = 12, 64
IN = 3072
CC = C // 128           # 6 channel chunks
TQ = 512                # tokens per core
KCN = T // 128          # 16 key chunks
ICN = IN // 128         # 24 inner chunks
HP = H // 2             # 6 head pairs
EPS = 1e-4
SCALE = 1.0 / np.sqrt(DH)

# param pack order in "prk" [128, CC, 8]
P_BQ, P_BK, P_BO, P_B2, P_L1S, P_L1B, P_L2S, P_L2B = range(8)


def _build_nc():
    nc = bacc.Bacc("TRN2", target_bir_lowering=False, debug=False,
                   enable_asserts=False, num_devices=8)
    d = {}
    d["xtq"] = nc.dram_tensor("xtq", [128, CC, TQ], F32, kind="ExternalInput").ap()
    d["xtqb"] = nc.dram_tensor("xtqb", [128, CC, TQ], BF16, kind="ExternalInput").ap()
    d["xtb"] = nc.dram_tensor("xtb", [128, CC, T], BF16, kind="ExternalInput").ap()
    for w, sh in (("wq", [128, CC, C]), ("wk", [128, CC, C]),
                  ("wv", [128, CC, C]), ("wo", [128, CC, C]),
                  ("w1", [128, CC, IN]), ("w2", [128, ICN, C])):
        d[w] = nc.dram_tensor(w, sh, BF16, kind="ExternalInput").ap()
    d["prk"] = nc.dram_tensor("prk", [128, CC, 8], F32, kind="ExternalInput").ap()
    d["b1p"] = nc.dram_tensor("b1p", [128, ICN], F32, kind="ExternalInput").ap()
    d["bvb"] = nc.dram_tensor("bvb", [128, C], F32, kind="ExternalInput").ap()
    d["msk"] = nc.dram_tensor("msk", [128, 2, 32], BF16, kind="ExternalInput").ap()
    d["ones"] = nc.dram_tensor("ones", [128, 128], F32R, kind="ExternalInput").ap()
    d["outT"] = nc.dram_tensor("outT", [128, CC, TQ], F32, kind="ExternalOutput").ap()

    with tile.TileContext(nc) as tc:
        _emit(nc, tc, d)
    nc.finalize()
    return nc


def _emit(nc, tc, d):
    # ---- persistent constants ------------------------------------------
    const = tc.alloc_tile_pool(name="const", bufs=1, side="left")
    ones_sb = const.tile([128, 128], F32R, name="ones_sb")
    prk_sb = const.tile([128, CC, 8], F32, name="prk_sb")
    b1p_sb = const.tile([128, ICN], F32, name="b1p_sb")
    bvb_sb = const.tile([128, C], F32, name="bvb_sb")
    msk_sb = const.tile([128, 2, 32], BF16, name="msk_sb")

    def prm(cc, pi):
        return prk_sb[:, cc, pi].unsqueeze(-1)  # [128,1]

    nc.sync.dma_start(out=prk_sb[:], in_=d["prk"][:])
    nc.sync.dma_start(out=msk_sb[:], in_=d["msk"][:])

    wo_pool = tc.alloc_tile_pool(name="wop", bufs=1, side="left")
    wo_t = wo_pool.tile([128, CC, C], BF16, name="wo_t")

    # right-side stack (LIFO): ctxT/xtq (A-C), qp/kv (A-B), xtqb/xtb (A)
    ctxT_pool = tc.alloc_tile_pool(name="ctxTp", bufs=1, side="right")
    ctxT_t = ctxT_pool.tile([128, CC, TQ], BF16, name="ctxT_t")
    xtq_pool = tc.alloc_tile_pool(name="xtq", bufs=1, side="right")
    xtq_t = xtq_pool.tile([128, CC, TQ], F32, name="xtq_t")
    qp_pool = tc.alloc_tile_pool(name="qp", bufs=1, side="right")
    qp_t = qp_pool.tile([128, CC, 2, TQ], BF16, name="qp_t")
    kv_pool = tc.alloc_tile_pool(name="kvp", bufs=1, side="right")
    kT_t = kv_pool.tile([128, CC, T], BF16, name="kT_t")
    v_t = kv_pool.tile([128, KCN, H, DH + 1], BF16, name="v_t")
    xtqb_pool = tc.alloc_tile_pool(name="xtqb", bufs=1, side="right")
    xtqb_t = xtqb_pool.tile([128, CC, TQ], BF16, name="xtqb_t")
    xtb_pool = tc.alloc_tile_pool(name="xtb", bufs=1, side="right")
    xtb_t = xtb_pool.tile([128, CC, T], BF16, name="xtb_t")

    # ==================== phase A: QKV ==================================
    with tc.tile_pool(name="wqkv", bufs=1, side="right") as wqkv, \
         tc.tile_pool(name="pqkv", bufs=2, space="PSUM") as pqkv:
        wq_t = wqkv.tile([128, CC, C], BF16, name="wq_t", tag="wq")
        wk_t = wqkv.tile([128, CC, C], BF16, name="wk_t", tag="wk")
        wv_t = wqkv.tile([128, CC, C], BF16, name="wv_t", tag="wv")
        nc.sync.dma_start(out=xtqb_t[:], in_=d["xtqb"][:])
        for i in range(3):
            nc.sync.dma_start(out=wq_t[:, 2 * i:2 * i + 2, :],
                              in_=d["wq"][:, 2 * i:2 * i + 2, :])
        nc.sync.dma_start(out=bvb_sb[:], in_=d["bvb"][:])
        nc.sync.dma_start(out=wv_t[:], in_=d["wv"][:])
        for i in range(3):
            nc.sync.dma_start(out=xtb_t[:, 2 * i:2 * i + 2, :],
                              in_=d["xtb"][:, 2 * i:2 * i + 2, :])
        nc.sync.dma_start(out=wk_t[:], in_=d["wk"][:])
        nc.sync.dma_start(out=ones_sb[:], in_=d["ones"][:])
        nc.sync.dma_start(out=b1p_sb[:], in_=d["b1p"][:])
        nc.sync.dma_start(out=xtq_t[:], in_=d["xtq"][:])

        # zero q staging (evicts below overwrite the live rows)
        nc.vector.memset(qp_t[:], 0.0)

        # warm up the gpsimd custom-instruction library off the critical
        # path (first dispatch pays a ~10us library-load/warmup penalty)
        gwarm = wqkv.tile([128, 16], F32R, name="gwarm", tag="gw")
        nc.gpsimd.partition_broadcast(gwarm[:], ones_sb[0:1, 0:16],
                                      channels=128)

        # q^T: per output chunk mc, heads 2mc (rows 0:64) / 2mc+1 (64:128)
        for mc in range(CC):
            ps = pqkv.tile([128, TQ], F32, name="ps_q", tag="pk", bufs=3)
            for kc in range(CC):
                nc.tensor.matmul(ps[:], wq_t[:, kc, mc * 128:(mc + 1) * 128],
                                 xtqb_t[:, kc, :],
                                 start=(kc == 0), stop=(kc == CC - 1))
            nc.vector.tensor_scalar_add(
                qp_t[0:64, mc, 0, :], ps[0:64, :], prm(mc, P_BQ)[0:64, :])
            nc.vector.tensor_scalar_add(
                qp_t[64:128, mc, 1, :], ps[64:128, :], prm(mc, P_BQ)[64:128, :])

        # v rows (token-major) + ones column for softmax denominators;
        # before k so the heavy v evicts drain while k matmuls run
        nc.vector.memset(v_t[:, :, :, DH], 1.0)
        for tch in range(KCN):
            ps1 = pqkv.tile([128, 512], F32, name="ps_v1", tag="pv1")
            ps2 = pqkv.tile([128, 256], F32, name="ps_v2", tag="pv2")
            for kc in range(CC):
                xsl = xtb_t[:, kc, tch * 128:(tch + 1) * 128]
                nc.tensor.matmul(ps1[:], xsl, wv_t[:, kc, 0:512],
                                 start=(kc == 0), stop=(kc == CC - 1))
                nc.tensor.matmul(ps2[:], xsl, wv_t[:, kc, 512:C],
                                 start=(kc == 0), stop=(kc == CC - 1))
            nc.vector.tensor_add(
                v_t[:, tch, 0:8, 0:DH],
                ps1[:].rearrange("p (h d) -> p h d", d=DH),
                bvb_sb[:, 0:512].rearrange("p (h d) -> p h d", d=DH))
            nc.vector.tensor_add(
                v_t[:, tch, 8:H, 0:DH],
                ps2[:].rearrange("p (h d) -> p h d", d=DH),
                bvb_sb[:, 512:C].rearrange("p (h d) -> p h d", d=DH))

        # k^T for the full 2048 keys, chunk mc ascending (attention pair 0
        # needs chunk 0 first)
        for mc in range(CC):
            for tb in range(4):
                ps = pqkv.tile([128, 512], F32, name="ps_k", tag="pk", bufs=3)
                for kc in range(CC):
                    nc.tensor.matmul(
                        ps[:], wk_t[:, kc, mc * 128:(mc + 1) * 128],
                        xtb_t[:, kc, tb * 512:(tb + 1) * 512],
                        start=(kc == 0), stop=(kc == CC - 1))
                nc.vector.tensor_scalar_add(
                    kT_t[:, mc, tb * 512:(tb + 1) * 512], ps[:],
                    prm(mc, P_BK))

    xtb_pool.release()
    xtqb_pool.release()

    # prefetch weights for later phases while attention runs (the pool
    # reuses the SBUF space xtb/xtqb just released)
    nc.sync.dma_start(out=wo_t[:], in_=d["wo"][:])
    wmlp_pool = tc.alloc_tile_pool(name="wmlp", bufs=1, side="left")
    w1_t = wmlp_pool.tile([128, CC, IN], BF16, name="w1_t")
    w2_t = wmlp_pool.tile([128, ICN, C], BF16, name="w2_t")

    # ==================== phase B: attention ============================
    with tc.tile_pool(name="attnp", bufs=4, side="right") as apool, \
         tc.tile_pool(name="psc", bufs=3, space="PSUM") as psc, \
         tc.tile_pool(name="pctx", bufs=1, space="PSUM") as pctx:

        def _score_chunk(hp, kc):
            """Scores + exp + mask for one key chunk. kc<8 uses 64-row PE
            tiling (both heads on independent array halves, different PSUM
            banks); kc>=8 one packed full-array matmul."""
            cc = hp
            q0 = 32 * kc
            w = TQ - q0
            ps = psc.tile([128, 2 * TQ], F32, name="ps_s", tag="s")
            et = apool.tile([128, 2 * TQ], BF16, name="et", tag="e",
                            bufs=KCN)
            if kc < 8:
                for j in range(2):
                    r0, off = j * DH, q0 if j == 0 else TQ
                    nc.tensor.matmul(
                        ps[:, off:off + w],
                        kT_t[r0:r0 + DH, cc, kc * 128:(kc + 1) * 128],
                        qp_t[r0:r0 + DH, hp, j, q0:],
                        start=True, stop=True)
                nc.scalar.activation(et[:, q0:TQ + w], ps[:, q0:TQ + w],
                                     ACTF.Exp, scale=float(SCALE))
                eview = et[:, q0:q0 + 2 * w].rearrange(
                    "p (j c) -> p j c", j=2)
                mv = [et[:, q0:TQ], et[:, TQ:TQ + w]]
            else:
                nc.tensor.matmul(ps[:, 0:2 * w],
                                 kT_t[:, cc, kc * 128:(kc + 1) * 128],
                                 qp_t[:, hp, :, q0:],
                                 start=True, stop=True)
                nc.scalar.activation(et[:, 0:2 * w], ps[:, 0:2 * w],
                                     ACTF.Exp, scale=float(SCALE))
                eview = et[:, 0:2 * w].rearrange(
                    "p (j c) -> p j c", j=2)
                mv = [et[:, j * w:(j + 1) * w] for j in range(2)]
            nc.vector.tensor_mul(eview[:, :, 0:32], eview[:, :, 0:32],
                                 msk_sb[:])
            return mv

        def _ctx_chunk(hp, kc, ets, ctx_live):
            q0 = 32 * kc
            for j in range(2):
                nc.tensor.matmul(ctx_live[j][:, q0:],
                                 v_t[:, kc, 2 * hp + j, :], ets[kc][j],
                                 start=(kc == 0), stop=(kc == KCN - 1))

        def _norm_ops(hp, ctx_live):
            ops = []
            for j in range(2):
                def _one(hp=hp, j=j, ctx_ps=ctx_live[j]):
                    cc, ro = hp, j * DH
                    # custom DVE/gpsimd ops need partition-0 operands
                    dnr = apool.tile([1, TQ], F32, name="dnr", tag="dnr",
                                     bufs=1)
                    nc.vector.tensor_copy(dnr[:], ctx_ps[DH:DH + 1, :])
                    rcp = apool.tile([1, TQ], F32, name="rcp", tag="rcp",
                                     bufs=1)
                    nc.vector.reciprocal_approx_fast(rcp[:], dnr[:])
                    bc = apool.tile([128, TQ], F32, name="bc", tag="bc",
                                    bufs=2)
                    nc.gpsimd.partition_broadcast(bc[:], rcp[:], channels=128)
                    # fused evict+normalize: (ctx * 1.0) * (1/denom)
                    nc.vector.scalar_tensor_tensor(
                        ctxT_t[ro:ro + DH, cc, :], ctx_ps[0:DH, :], 1.0,
                        bc[ro:ro + DH, :],
                        mybir.AluOpType.mult, mybir.AluOpType.mult)
                ops.append(_one)
            return ops

        # per pair: the 16 64-mode score matmuls run first (one PE-mode
        # switch), then ctx chunks interleave with the packed kc>=8 scores
        # (all full-array) so the PE never waits on the exp stream
        pending = []
        for hp in range(HP):
            # spread the 9.4MB W1/W2 prefetch across the pair loop so the
            # HBM burst doesn't collide with attention (and, across cores,
            # with itself)
            if hp < 3:
                nc.sync.dma_start(out=w1_t[:, 2 * hp:2 * hp + 2, :],
                                  in_=d["w1"][:, 2 * hp:2 * hp + 2, :])
            else:
                g = hp - 3
                nc.sync.dma_start(out=w2_t[:, 8 * g:8 * g + 8, :],
                                  in_=d["w2"][:, 8 * g:8 * g + 8, :])
            ctx_live = [pctx.tile([DH + 1, TQ], F32, name=f"ctx{j}",
                                  tag=f"ctx{j}") for j in range(2)]
            ets = {}
            if pending:
                pending.pop(0)()
            for kc in range(8):
                ets[kc] = _score_chunk(hp, kc)
            if pending:
                pending.pop(0)()
            for i in range(8):
                ets[8 + i] = _score_chunk(hp, 8 + i)
                _ctx_chunk(hp, i, ets, ctx_live)
            for kc in range(8, KCN):
                _ctx_chunk(hp, kc, ets, ctx_live)
            pending.extend(_norm_ops(hp, ctx_live))
        for op in pending:
            op()

    kv_pool.release()
    qp_pool.release()

    # ==================== phase C: Wo + residual + LN1 ==================
    hT_holder = {}
    with tc.tile_pool(name="cpool", bufs=2, side="right") as cpool, \
         tc.tile_pool(name="r1pool", bufs=1, side="right") as r1pool:
        r1_t = r1pool.tile([128, CC, TQ], F32R, name="r1_t")
        with tc.tile_pool(name="pao", bufs=2, space="PSUM") as pao, \
             tc.tile_pool(name="pst", bufs=2, space="PSUM") as pst:
            ps_sum = pst.tile([1, TQ], F32, name="ps_sum", tag="st")
            ps_sq = pst.tile([1, TQ], F32, name="ps_sq", tag="st")
            sqs = []
            for mc in range(CC):
                ps = pao.tile([128, TQ], F32, name="ps_ao", tag="ao")
                for kc in range(CC):
                    nc.tensor.matmul(ps[:],
                                     wo_t[:, kc, mc * 128:(mc + 1) * 128],
                                     ctxT_t[:, kc, :],
                                     start=(kc == 0), stop=(kc == CC - 1))
                nc.vector.scalar_tensor_tensor(
                    r1_t[:, mc, :], ps[:], prm(mc, P_BO), xtq_t[:, mc, :],
                    mybir.AluOpType.add, mybir.AluOpType.add)
                sq = cpool.tile([128, TQ], F32R, name="sq", tag="sq", bufs=CC)
                nc.scalar.activation(sq[:], r1_t[:, mc, :], ACTF.Square)
                sqs.append(sq)
            # stats matmuls batched at the end: one PE mode-switch region
            for mc in range(CC):
                nc.tensor.matmul(ps_sum[:], ones_sb[:, 0:1], r1_t[:, mc, :],
                                 start=(mc == 0), stop=(mc == CC - 1))
            for mc in range(CC):
                nc.tensor.matmul(ps_sq[:], ones_sb[:, 0:1], sqs[mc][:],
                                 start=(mc == 0), stop=(mc == CC - 1))
            mean1, ex21 = _ln_stats(nc, cpool, ps_sum, ps_sq, "l1")
        hT_pool = tc.alloc_tile_pool(name="hTp", bufs=1, side="left")
        hTf_t = hT_pool.tile([128, CC, TQ], F32R, name="hTf_t")
        hTb_t = hT_pool.tile([128, CC, TQ], BF16, name="hTb_t")
        hT_holder["pool"] = hT_pool
        _ln_apply(nc, cpool, ones_sb, mean1, ex21,
                  lambda cc: r1_t[:, cc, :],
                  lambda cc: hTf_t[:, cc, :],
                  [prm(cc, P_L1S) for cc in range(CC)],
                  [prm(cc, P_L1B) for cc in range(CC)], "l1")
        for cc in range(CC):
            nc.vector.tensor_copy(hTb_t[:, cc, :], hTf_t[:, cc, :])

    xtq_pool.release()
    ctxT_pool.release()

    # ==================== phase D: MLP + residual + LN2 =================
    with tc.tile_pool(name="dpool", bufs=3, side="right") as dpool, \
         tc.tile_pool(name="r2pool", bufs=1, side="right") as r2pool:
        r2_t = r2pool.tile([128, CC, TQ], F32R, name="r2_t")
        with tc.tile_pool(name="pfc2", bufs=1, space="PSUM") as pfc2:
            ps_m = [pfc2.tile([128, TQ], F32, name=f"ps_m{mc}", tag=f"m{mc}")
                    for mc in range(CC)]
            with tc.tile_pool(name="pfc1", bufs=2, space="PSUM") as pfc1:
                for ic in range(ICN):
                    ps1 = pfc1.tile([128, TQ], F32, name="ps1", tag="f1")
                    for kc in range(CC):
                        nc.tensor.matmul(
                            ps1[:], w1_t[:, kc, ic * 128:(ic + 1) * 128],
                            hTb_t[:, kc, :],
                            start=(kc == 0), stop=(kc == CC - 1))
                    g = dpool.tile([128, TQ], BF16, name="g", tag="g")
                    nc.scalar.activation(g[:], ps1[:], ACTF.Gelu_apprx_tanh,
                                         bias=b1p_sb[:, ic].unsqueeze(-1))
                    for mc in range(CC):
                        nc.tensor.matmul(ps_m[mc][:],
                                         w2_t[:, ic, mc * 128:(mc + 1) * 128],
                                         g[:], start=(ic == 0),
                                         stop=(ic == ICN - 1))
            with tc.tile_pool(name="pst2", bufs=2, space="PSUM") as pst2:
                ps_sum2 = pst2.tile([1, TQ], F32, name="ps_sum2", tag="st")
                ps_sq2 = pst2.tile([1, TQ], F32, name="ps_sq2", tag="st")
                sqs2 = []
                for mc in range(CC):
                    nc.vector.scalar_tensor_tensor(
                        r2_t[:, mc, :], ps_m[mc][:], prm(mc, P_B2),
                        hTf_t[:, mc, :], mybir.AluOpType.add,
                        mybir.AluOpType.add)
                    sq = dpool.tile([128, TQ], F32R, name="sq2", tag="sq",
                                    bufs=CC)
                    nc.scalar.activation(sq[:], r2_t[:, mc, :], ACTF.Square)
                    sqs2.append(sq)
                for mc in range(CC):
                    nc.tensor.matmul(ps_sum2[:], ones_sb[:, 0:1],
                                     r2_t[:, mc, :],
                                     start=(mc == 0), stop=(mc == CC - 1))
                for mc in range(CC):
                    nc.tensor.matmul(ps_sq2[:], ones_sb[:, 0:1], sqs2[mc][:],
                                     start=(mc == 0), stop=(mc == CC - 1))
                mean2, ex22 = _ln_stats(nc, dpool, ps_sum2, ps_sq2, "l2")
        hT_holder["pool"].release()
        _ln_apply(nc, dpool, ones_sb, mean2, ex22,
                  lambda cc: r2_t[:, cc, :],
                  lambda cc: dpool.tile([128, TQ], F32, name="o",
                                        tag="o", bufs=2)[:],
                  [prm(cc, P_L2S) for cc in range(CC)],
                  [prm(cc, P_L2B) for cc in range(CC)], "l2",
                  post_f=lambda cc, ap: nc.sync.dma_start(
                      out=d["outT"][:, cc, :], in_=ap))

    wmlp_pool.release()
    wo_pool.release()
    const.release()


def _ln_stats(nc, pool, ps_sum, ps_sq, tagp):
    n = float(C)
    mean = pool.tile([1, TQ], F32, name="ln_mean", tag=tagp + "mean")
    nc.scalar.activation(mean[:], ps_sum[:], ACTF.Copy, scale=1.0 / n)
    ex2 = pool.tile([1, TQ], F32, name="ln_ex2", tag=tagp + "ex2")
    nc.scalar.activation(ex2[:], ps_sq[:], ACTF.Copy, scale=1.0 / n)
    return mean, ex2


def _ln_apply(nc, pool, ones_sb, mean, ex2, in_f, out_f,
              scales, biases, tagp, post_f=None):
    """out = (in - mean)/sqrt(var_unbiased + eps) * s + b, stats over C."""
    n = float(C)
    m2 = pool.tile([1, TQ], F32, name="ln_m2", tag=tagp + "m2", bufs=1)
    nc.vector.tensor_mul(m2[:], mean[:], mean[:])
    dv = pool.tile([1, TQ], F32, name="ln_d", tag=tagp + "d", bufs=1)
    nc.vector.tensor_sub(dv[:], ex2[:], m2[:])
    eps_sb = pool.tile([1, 1], F32, name="ln_eps", tag=tagp + "eps", bufs=1)
    nc.vector.memset(eps_sb[:], float(EPS))
    std = pool.tile([1, TQ], F32, name="ln_std", tag=tagp + "std", bufs=1)
    nc.scalar.activation(std[:], dv[:], ACTF.Sqrt,
                         scale=n / (n - 1.0), bias=eps_sb[:])
    istd = pool.tile([1, TQ], F32, name="ln_istd", tag=tagp + "istd", bufs=1)
    nc.vector.reciprocal_approx_fast(istd[:], std[:])
    # broadcast mean and istd across partitions on the (idle) gpsimd engine
    mb = pool.tile([128, TQ], F32, name="ln_mb", tag=tagp + "mb", bufs=1)
    nc.gpsimd.partition_broadcast(mb[:], mean[:], channels=128)
    ib = pool.tile([128, TQ], F32, name="ln_ib", tag=tagp + "ib", bufs=1)
    nc.gpsimd.partition_broadcast(ib[:], istd[:], channels=128)
    for cc in range(CC):
        t1 = pool.tile([128, TQ], F32, name="ln_t1", tag=tagp + "t1", bufs=2)
        nc.vector.tensor_sub(t1[:], in_f(cc), mb[:])
        t2 = pool.tile([128, TQ], F32, name="ln_t2", tag=tagp + "t2", bufs=2)
        nc.vector.tensor_mul(t2[:], t1[:], ib[:])
        out_ap = out_f(cc)
        nc.vector.tensor_scalar(out_ap, t2[:], scales[cc], biases[cc],
                                mybir.AluOpType.mult, mybir.AluOpType.add)
        if post_f is not None:
            post_f(cc, out_ap)


_NC = None


def _get_nc():
    global _NC
    if _NC is None:
        _NC = _build_nc()
    return _NC


def _prep_inmaps(x, Wq, bq, Wk, bk, Wv, bv, Wo, bo, ln1_s, ln1_b,
                 W1, b1, W2, b2, ln2_s, ln2_b):
    f32 = np.float32
    bf16 = ml_dtypes.bfloat16

    def wpack(w, nch, width):
        # [nch*128, width] -> [128, nch, width]
        return np.ascontiguousarray(
            np.asarray(w, f32).reshape(nch, 128, width).transpose(1, 0, 2)
        ).astype(bf16)

    wq = wpack(Wq, CC, C)
    wk = wpack(Wk, CC, C)
    wv = wpack(Wv, CC, C)
    wo = wpack(Wo, CC, C)
    w1 = wpack(W1, CC, IN)
    w2 = wpack(W2, ICN, C)
    prk = np.zeros((128, CC, 8), f32)
    for pi, arr in ((P_BQ, bq), (P_BK, bk), (P_BO, bo), (P_B2, b2),
                    (P_L1S, ln1_s), (P_L1B, ln1_b), (P_L2S, ln2_s),
                    (P_L2B, ln2_b)):
        prk[:, :, pi] = np.asarray(arr, f32).reshape(CC, 128).T
    b1p = np.ascontiguousarray(np.asarray(b1, f32).reshape(ICN, 128).T)
    bvb = np.broadcast_to(np.asarray(bv, f32)[None, :], (128, C)).copy()
    ones = np.ones((128, 128), f32)

    xT = [np.ascontiguousarray(np.asarray(x)[b].T, dtype=f32)
          for b in range(B)]
    kk = np.arange(128)[:, None]
    qq = np.arange(32)[None, :]
    in_maps = []
    for c in range(8):
        b, p = c // 4, c % 4
        xtq = np.ascontiguousarray(
            xT[b][:, p::4].reshape(CC, 128, TQ).transpose(1, 0, 2))
        xtb = np.ascontiguousarray(
            xT[b].reshape(CC, 128, T).transpose(1, 0, 2)).astype(bf16)
        msk = np.zeros((128, 2, 32), bf16)
        msk[:, 0, :] = (kk <= 4 * qq + p).astype(bf16)
        msk[:, 1, :] = msk[:, 0, :]
        in_maps.append({
            "xtq": xtq, "xtqb": xtq.astype(bf16), "xtb": xtb,
            "wq": wq, "wk": wk, "wv": wv, "wo": wo, "w1": w1, "w2": w2,
            "prk": prk, "b1p": b1p, "bvb": bvb, "msk": msk, "ones": ones,
        })
    return in_maps


def _run(in_maps, trace=False, **kw):
    nc = _get_nc()
    return run_bass_kernel_spmd(nc, in_maps, list(range(8)), trace=trace, **kw)


def kernel(**inputs):
    in_maps = _prep_inmaps(**inputs)
    res = _run(in_maps)
    out = np.empty((B, T, C), np.float32)
    for c in range(8):
        b, p = c // 4, c % 4
        o = res.results[c]["outT"]  # [128, CC, TQ]
        out[b, p::4, :] = o.transpose(1, 0, 2).reshape(C, TQ).T
    return out
